# revision 38
# baseline (speedup 1.0000x reference)
"""Multi-head attention (B=4, S=2048, D=768, H=12) on 8 TRN2 NeuronCores.

Sharding: core i -> batch i//2, heads 6*(i%2) .. 6*(i%2)+6 (48 (b,h) pairs,
6 per core). Each core computes q^T/k^T in [d, s] layout, V in natural
[s, d] layout (bf16, with an appended ones-column so the softmax denominator
falls out of the attnV matmul), transposed scores S^T[k, q], exp on the
scalar engine (bf16 out), then the partial output projection over its 384
attention-output channels. The two cores sharing a batch have their partial
projections summed host-side, which stands in for the tensor-parallel
all-reduce.

Performance structure:
- Matmuls in f32r (TF32-like, 1 cyc/row at N>=256); paired score matmuls in
  64-row groups; attention inner loop software-pipelined (attnV trails the
  exp stream by TRAIL k-blocks; divisions of head pair j run inside pair
  j+1's loop).
- Attention processed per (head-pair, 512-wide q-quarter) so every psum
  tile is one bank: 6 rotating score tiles + 2 attnV accumulators. HW
  cross-engine dependency latency is far above the cost model's, so deep
  score rotation (3 k-blocks of slack on the scores->exp handoff) is worth
  more than big tiles (HW-swept: 6/2 = 434us vs 4/4 = 465us).
- exp split across engines: head A + first 3 k-blocks on ACT (table exp),
  head B's k-blocks 3-15 on the DVE via Schraudolph int16-bitcast
  (A_SCH*s + B_SCH -> bf16 bits), keeping ACT under the PE's per-k-block
  budget at ~+0.75% attention-output error (gate is 2e-2).
- Softmax denominator via an appended ones-column in V; normalize chain
  reciprocal (DVE) -> partition_broadcast (gpsimd) -> multiply (DVE), no
  PE matmul, issued two k-blocks early through the carry pipeline.
- x^T / w_qkv / w_v inputs in bf16: halves per-rep input DMA (9.8 -> 4.9 MB)
  and SBUF footprint; V/expS tiles bf16. Input DMAs split across both
  HWDGE queues (SP+ACT), first chunks small so the first matmul starts
  ~2us earlier; exp-table warm + bqk load issued after the critical DMAs.
- V-projection bias folded to the host (softmax rows sum to 1):
  y += b_v @ w_out + b_out in finish_output().
- Phase-B psum pool at bufs=8; phase D has its own 4-buffer psum pool and
  runs sb 8-15 first so the last pairs' div chains hide behind it; output
  staging + y DMA in bf16 (partials upcast to fp32 on the host).
- Timing methodology: wall-clock through the axon tunnel is dominated by
  per-call payload (~30MB/s, +-0.3s jitter), so exec time is measured with
  _build(hw_reps=K): a sequencer For_i loop around the body (constant NEFF
  size); (t[K=4001] - t[K=1]) / 4000 resolves per-rep time to a few us
  (bench3.py).
"""

import os

import numpy as np

import concourse.bass as bass
from concourse import bacc
import concourse.mybir as mybir
import concourse.tile as tile
from concourse.bass_utils import run_bass_kernel_spmd

F32 = mybir.dt.float32
F32R = mybir.dt.float32r
BF16 = mybir.dt.bfloat16
I16 = mybir.dt.int16
AF = mybir.ActivationFunctionType
ALU = mybir.AluOpType

B, S, D = 4, 2048, 768
H, HD = 12, 64
HPC = 6            # heads per core
GC = HPC * HD      # 384 channels per core
N_CORES = 8
SCALE = 1.0 / np.sqrt(np.float32(H))   # NOTE: reference scales by 1/sqrt(H)

# Schraudolph constants: bits = A_SCH * s_raw + B_SCH, int16 -> bitcast bf16
A_SCH = float(128.0 * np.log2(np.e) * SCALE)
B_SCH = float(128.0 * 127.0 - 7.5)

SPLIT_ATTNV = False
SPLIT_PROJ = False
SPLIT_OUTPROJ = False
# k-blocks whose second-head (hi==1) exp runs on the DVE (Schraudolph)
# instead of the scalar engine: unloads ACT (the phase-C co-critical engine)
# at ~1% extra attention-output error (validated in numpy; gate is 2e-2).
# Head A's exp always stays on ACT; kb 0-2 also stay on ACT so the DVE is
# clear of the div-carry burst (reciprocal) at the start of each pair.
DVE_KBS = frozenset(range(3, 16))
if os.environ.get("KNOB_DVE_KBS") is not None:
    _v = os.environ["KNOB_DVE_KBS"]
    DVE_KBS = frozenset(int(x) for x in _v.split(",") if x != "")
SKIP = frozenset()
# psum split: 6 single-bank score tiles (3-kb rotation slack on the
# scores->exp handoff, the dominant HW stall) + 2 attnV accumulators.
# HW-swept 4/4=465us, 5/3=441us, 6/2=434us on the For_i paired bench.
PSS_BUFS = int(os.environ.get("KNOB_PSS_BUFS", "6"))
PSO_BUFS = int(os.environ.get("KNOB_PSO_BUFS", "2"))

_NC_CACHE = None
LAST_RESULTS = None


def _build(reps=1, hw_reps=1):
    """reps: python-unrolled repetitions (NEFF grows per rep).
    hw_reps: sequencer-level For_i loop around the body (constant NEFF size;
    used for tunnel-immune on-device timing via large iteration counts)."""
    nc = bacc.Bacc("TRN2", target_bir_lowering=False, debug=False,
                   num_devices=N_CORES)
    xt = nc.dram_tensor("xt", (D, S), BF16, kind="ExternalInput")
    wqk = nc.dram_tensor("wqk", (D, 2 * GC), BF16, kind="ExternalInput")
    wv = nc.dram_tensor("wv", (D, GC), BF16, kind="ExternalInput")
    bqk = nc.dram_tensor("bqk", (2 * GC,), F32, kind="ExternalInput")
    wout = nc.dram_tensor("wout", (GC, D), F32, kind="ExternalInput")
    y = nc.dram_tensor("y", (S, D), BF16, kind="ExternalOutput")

    NSB = S // 128        # 16 s-blocks
    NDS = D // 128        # 6 d-subtiles
    NOB = 2 * GC // 128   # 6 q+k output blocks
    NCS = GC // 128       # 3 c-subtiles for out-proj

    with tile.TileContext(nc) as tc:
        with (
            tc.tile_pool(name="const", bufs=1) as cpool,
            tc.tile_pool(name="bigqv", bufs=1) as bigqv,
            tc.tile_pool(name="bigd", bufs=1) as bigd,
        ):
            ones_sb = cpool.tile([1, 128], F32)
            nc.gpsimd.memset(ones_sb[:], 1.0)
            ones_r = cpool.tile([1, 128], F32R)
            nc.vector.tensor_copy(ones_r[:], ones_sb[:])
            # exp ACT table preload + bqk load are emitted inside rep_body
            # AFTER the phase-B input-DMA dispatches: both sit on queues
            # (ACT/SP) whose first dma_starts gate the very first matmul
            warm = cpool.tile([1, 8], F32)
            bqk_sb = cpool.tile([128, NOB], F32)
            wout_sb = cpool.tile([128, NCS, D], F32R)

            def warm_and_bqk():
                nc.sync.dma_start(
                    bqk_sb[:], bqk.ap().rearrange("(ob p) -> p ob", p=128))
                nc.scalar.activation(warm[:], ones_sb[:, :8], AF.Exp)

            qkT = bigqv.tile([128, NOB, S], F32R)     # blocks 0-2 q^T, 3-5 k^T
            V_sb = bigqv.tile([128, NSB, HPC * (HD + 1)], BF16)  # V + ones col
            attnT = bigd.tile([128, NCS, S], F32R)    # attention out, [c, s]

            V_view = V_sb[:].rearrange("p b (h e) -> p b h e", e=HD + 1)
            ones_col = cpool.tile([128, 1], BF16)
            nc.gpsimd.memset(ones_col[:], 1.0)
            nc.vector.tensor_copy(
                V_view[:, :, :, HD], ones_col[:, :, None].to_broadcast([128, NSB, HPC])
            )

            xt_src = xt.ap().rearrange("(ds p) s -> p ds s", p=128)
            wqk_src = wqk.ap().rearrange("(ds p) o -> p ds o", p=128)

            rep_ctx = tc.tile_pool(name="xtp", bufs=2)
            xtp = rep_ctx.__enter__()

            def rep_body(load_wout):
                # ---- Phase B: projections ----
                OB_ORDER = (0, 3, 1, 4, 2, 5)
                with (
                    tc.tile_pool(name="psb", bufs=8, space="PSUM") as psb,
                ):
                    xt_sb = xtp.tile([128, NDS, S], BF16, tag="xt",
                                     name="xt_sb")
                    wqk_sb = xtp.tile([128, NDS, 2 * GC], BF16, tag="wq",
                                      name="wqk_sb")
                    wv_sb = xtp.tile([128, NDS, GC], BF16, tag="wv",
                                     name="wv_sb")
                    # input DMAs split across the two HWDGE queues (SP+ACT,
                    # ~600ns descriptor-gen each, serial per queue); xt's
                    # first chunk in ds-halves so the first ob-0 matmuls
                    # (ds-major) start as early as possible
                    nc.scalar.dma_start(
                        wqk_sb[:, :, 0:128], wqk_src[:, :, 0:128],
                    )
                    for d0, d1 in ((0, 3), (3, 6)):
                        nc.sync.dma_start(
                            xt_sb[:, d0:d1, 0:512],
                            xt_src[:, d0:d1, 0:512],
                        )
                    nc.scalar.dma_start(
                        xt_sb[:, :, 512:1024], xt_src[:, :, 512:1024],
                    )
                    nc.scalar.dma_start(
                        wqk_sb[:, :, 3 * 128:4 * 128],
                        wqk_src[:, :, 3 * 128:4 * 128],
                    )
                    for sc, eng in ((2, nc.sync), (3, nc.scalar)):
                        eng.dma_start(
                            xt_sb[:, :, sc * 512:(sc + 1) * 512],
                            xt_src[:, :, sc * 512:(sc + 1) * 512],
                        )
                    if load_wout:
                        warm_and_bqk()
                    for ob in (1, 4, 2, 5):
                        nc.sync.dma_start(
                            wqk_sb[:, :, ob * 128:(ob + 1) * 128],
                            wqk_src[:, :, ob * 128:(ob + 1) * 128],
                        )
                    nc.sync.dma_start(
                        wv_sb[:],
                        wv.ap().rearrange("(ds p) o -> p ds o", p=128),
                    )
                    if load_wout:
                        nc.sync.dma_start(
                            wout_sb[:],
                            wout.ap().rearrange(
                                "(cs p) o -> p cs o", p=128).bitcast(F32R),
                        )

                    # q^T / k^T: [o, s] = wqk^T @ x^T
                    halves = ((0, 64), (64, 128)) if SPLIT_PROJ else ((0, 128),)
                    for ob in OB_ORDER:
                        pss4 = [psb.tile([128, 512], F32, tag="ps",
                                         name=f"ps{ob}_{sc}") for sc in range(4)]
                        # ob 0 runs sc-outer: its first 6 matmuls then need
                        # only the first xt chunk, which lands ~3us before
                        # the rest of xt
                        if ob == 0:
                            loop_iter = [(ds, sc) for sc in range(4)
                                         for ds in range(NDS)]
                        else:
                            loop_iter = [(ds, sc) for ds in range(NDS)
                                         for sc in range(4)]
                        for ds, sc in loop_iter if "proj" not in SKIP else ():
                            for hi, (r0, r1) in enumerate(halves):
                                nc.tensor.matmul(
                                    pss4[sc][:],
                                    wqk_sb[r0:r1, ds, ob * 128:(ob + 1) * 128],
                                    xt_sb[r0:r1, ds, sc * 512:(sc + 1) * 512],
                                    start=(ds == 0 and hi == 0),
                                    stop=(ds == NDS - 1
                                          and hi == len(halves) - 1),
                                    skip_group_check=SPLIT_PROJ,
                                )
                        for sc in range(4):
                            nc.vector.tensor_scalar_add(
                                qkT[:, ob, sc * 512:(sc + 1) * 512], pss4[sc][:],
                                bqk_sb[:, ob:ob + 1],
                            )

                    # V natural: [s, o] = x @ wv   (bias folded to host)
                    for sb in range(NSB):
                        ps = psb.tile([128, 512], F32, tag="ps")
                        for ds in range(NDS) if "proj" not in SKIP else ():
                            for hi, (r0, r1) in enumerate(halves):
                                nc.tensor.matmul(
                                    ps[:, :GC],
                                    xt_sb[r0:r1, ds, sb * 128:(sb + 1) * 128],
                                    wv_sb[r0:r1, ds, :],
                                    start=(ds == 0 and hi == 0),
                                    stop=(ds == NDS - 1
                                          and hi == len(halves) - 1),
                                    skip_group_check=True,
                                )
                        nc.vector.tensor_copy(V_view[:, sb, :, 0:HD], ps[:, :GC])

                # ---- Phase C: attention per (head-pair, q-quarter) ----
                # q processed in 512-wide quarters so every psum tile is a
                # single bank: pss 4 bufs + pso 4 bufs fills the 8 banks and
                # doubles the rotation slack on every cross-engine handoff
                # (HW dependency latency is far above the cost model's 100ns
                # -- measured via SKIP ablations on the For_i bench).
                with (
                    tc.tile_pool(name="bigc", bufs=1) as bigc,
                    tc.tile_pool(name="cw", bufs=1) as cw,
                    tc.tile_pool(name="pss", bufs=PSS_BUFS, space="PSUM") as pss,
                    tc.tile_pool(name="pso", bufs=PSO_BUFS, space="PSUM") as pso,
                ):
                    QQ = 512
                    NQQ = S // QQ  # 4 q-quarters
                    NBUF = 12      # rotating S^T exp slots (2 per k-block)
                    expS = bigc.tile([128, NBUF, QQ], BF16)

                    def slot(kb, hi):
                        return (2 * kb + hi) % NBUF

                    def make_div(ps_o, h, qq):
                        # normalize out'[d, q] by Z[q] (ones-column row).
                        # reciprocal [1,QQ] (DVE, from psum), partition-
                        # broadcast (gpsimd), multiply (DVE): no PE matmul,
                        # no shared-psum-pool collision with the score tiles.
                        # Split into two stages so both heads' reciprocals and
                        # broadcasts issue before either multiply (in-order
                        # engine queues), letting ps_o free as early as
                        # possible for the next pair's attnV.
                        if "div" in SKIP:
                            return lambda: None, lambda: None
                        base = (h % 2) * 64
                        qob = h // 2
                        rz = cw.tile([1, QQ], F32, tag="rz", bufs=2, name="rz")
                        rzb_sb = cw.tile([64, QQ], F32, tag="rzb", bufs=2,
                                         name="rzb_sb")

                        def recip_bcast():
                            with nc.allow_low_precision(reason="f32r softmax denom"):
                                nc.vector.reciprocal(rz[:], ps_o[HD:HD + 1, :])
                            nc.gpsimd.partition_broadcast(rzb_sb[:], rz[:])

                        def norm():
                            nc.vector.tensor_mul(
                                attnT[base:base + 64, qob, qq * QQ:(qq + 1) * QQ],
                                ps_o[0:HD, :], rzb_sb[:],
                            )
                        return recip_bcast, norm

                    vhalves = ((0, 64), (64, 128)) if SPLIT_ATTNV else ((0, 128),)

                    def attn_v(ps_o, h, kb, start, stop):
                        if "attnv" in SKIP:
                            return
                        sl = slot(kb, h % 2)
                        for hi, (r0, r1) in enumerate(vhalves):
                            nc.tensor.matmul(
                                ps_o[:, :],
                                V_sb[r0:r1, kb,
                                     h * (HD + 1):(h + 1) * (HD + 1)],
                                expS[r0:r1, sl, :],
                                start=(start and hi == 0),
                                stop=(stop and hi == len(vhalves) - 1),
                                skip_group_check=True,
                            )

                    def do_exp(ps_s, kb, hi):
                        if "exp" in SKIP:
                            return
                        sl = slot(kb, hi)
                        if hi == 1 and kb in DVE_KBS:
                            nc.vector.tensor_scalar(
                                expS[:, sl, :].bitcast(I16), ps_s[:],
                                A_SCH, B_SCH, ALU.mult, ALU.add,
                            )
                        else:
                            nc.scalar.activation(
                                expS[:, sl, :], ps_s[:], AF.Exp,
                                scale=float(SCALE),
                            )

                    TRAIL = 3
                    carry = []   # closures from the previous (pair, qq)
                    # qq 2,3 first: phase D then starts with sb 8-15
                    # (ready long before), hiding the last pairs' div chains
                    for qq in (2, 3, 0, 1):
                        for hp in range(HPC // 2):
                            hA, hB = 2 * hp, 2 * hp + 1
                            qob = hp
                            kob = NCS + hp
                            ps_oA = pso.tile([HD + 1, QQ], F32, tag="o", name="ps_oA")
                            ps_oB = pso.tile([HD + 1, QQ], F32, tag="o", name="ps_oB")
                            for kb in range(NSB):
                                ps_sA = pss.tile([128, QQ], F32, tag="s", name="ps_sA")
                                ps_sB = pss.tile([128, QQ], F32, tag="s", name="ps_sB")
                                for base, ps_s in ((0, ps_sA), (64, ps_sB)) \
                                        if "scores" not in SKIP else ():
                                    nc.tensor.matmul(
                                        ps_s[:],
                                        qkT[base:base + 64, kob,
                                            kb * 128:(kb + 1) * 128],
                                        qkT[base:base + 64, qob,
                                            qq * QQ:(qq + 1) * QQ],
                                        start=True, stop=True,
                                    )
                                do_exp(ps_sA, kb, 0)
                                do_exp(ps_sB, kb, 1)
                                if kb < len(carry):
                                    carry[kb]()
                                if kb >= TRAIL:
                                    pk = kb - TRAIL
                                    attn_v(ps_oA, hA, pk, start=(pk == 0), stop=False)
                                    attn_v(ps_oB, hB, pk, start=(pk == 0), stop=False)
                            for pk in range(NSB - TRAIL, NSB - 1):
                                attn_v(ps_oA, hA, pk, start=False, stop=False)
                                attn_v(ps_oB, hB, pk, start=False, stop=False)
                            rbA, normA = make_div(ps_oA, hA, qq)
                            rbB, normB = make_div(ps_oB, hB, qq)
                            carry = [
                                lambda a=ps_oA, b=ps_oB, h1=hA, h2=hB, \
                                        rA=rbA, rB=rbB: (
                                    attn_v(a, h1, NSB - 1, start=False, stop=True),
                                    attn_v(b, h2, NSB - 1, start=False, stop=True),
                                    rA(), rB(),
                                ),
                                lambda nA=normA, nB=normB: (nA(), nB()),
                            ]
                    for f in carry:
                        f()

                # ---- Phase D: output projection (partial, 384 c) ----
                # own psum pool (phase C's pools are closed); sb 8-15 first
                # (qq 2,3 attnT, normalized pairs ago) so the last pairs'
                # div chains hide behind them
                with (
                    tc.tile_pool(name="psd", bufs=4, space="PSUM") as psd,
                    tc.tile_pool(name="cwd", bufs=1) as cwd,
                ):
                    chalves = ((0, 64), (64, 128)) if SPLIT_OUTPROJ else ((0, 128),)
                    for sb in (*range(8, NSB), *range(8)):
                        ps_f = psd.tile([128, D], F32, tag="f", name="ps_f")
                        for cs in range(NCS) if "outproj" not in SKIP else ():
                            for hi, (r0, r1) in enumerate(chalves):
                                for o0, n in ((0, 512), (512, 256)):
                                    nc.tensor.matmul(
                                        ps_f[:, o0:o0 + n],
                                        attnT[r0:r1, cs, sb * 128:(sb + 1) * 128],
                                        wout_sb[r0:r1, cs, o0:o0 + n],
                                        start=(cs == 0 and hi == 0),
                                        stop=(cs == NCS - 1
                                              and hi == len(chalves) - 1),
                                        skip_group_check=SPLIT_OUTPROJ,
                                    )
                        if "outproj" not in SKIP:
                            ostage = cwd.tile([128, D], BF16, tag="ostage",
                                              bufs=3, name="ostage")
                            nc.any.tensor_copy(ostage[:], ps_f[:, :D])
                            nc.sync.dma_start(
                                y.ap()[sb * 128:(sb + 1) * 128, :], ostage[:])

            if hw_reps > 1:
                warm_and_bqk()
                nc.sync.dma_start(
                    wout_sb[:],
                    wout.ap().rearrange("(cs p) o -> p cs o", p=128).bitcast(F32R),
                )
                with tc.For_i(0, hw_reps):
                    rep_body(load_wout=False)
            else:
                for _rep in range(reps):
                    rep_body(load_wout=(_rep == 0))
            rep_ctx.__exit__(None, None, None)

    nc.compile()
    return nc


def _get_nc():
    global _NC_CACHE
    if _NC_CACHE is None:
        _NC_CACHE = _build()
    return _NC_CACHE


def make_in_maps(x, w_qkv, b_qkv, w_out, b_out):
    x = np.asarray(x, dtype=np.float32)
    w_qkv = np.asarray(w_qkv, dtype=np.float32)
    b_qkv = np.asarray(b_qkv, dtype=np.float32)
    w_out = np.asarray(w_out, dtype=np.float32)

    in_maps = []
    for i in range(N_CORES):
        b = i // 2
        c0 = (i % 2) * GC
        q_sl = slice(c0, c0 + GC)
        k_sl = slice(D + c0, D + c0 + GC)
        v_sl = slice(2 * D + c0, 2 * D + c0 + GC)
        import ml_dtypes
        bf = ml_dtypes.bfloat16
        in_maps.append({
            "xt": np.ascontiguousarray(x[b].T.astype(bf)),
            "wqk": np.ascontiguousarray(np.concatenate(
                [w_qkv[:, q_sl], w_qkv[:, k_sl]], axis=1).astype(bf)),
            "wv": np.ascontiguousarray(w_qkv[:, v_sl].astype(bf)),
            "bqk": np.ascontiguousarray(
                np.concatenate([b_qkv[q_sl], b_qkv[k_sl]])),
            "wout": np.ascontiguousarray(w_out[c0:c0 + GC, :]),
        })
    return in_maps


def finish_output(res, x, w_qkv, b_qkv, w_out, b_out):
    b_qkv = np.asarray(b_qkv, dtype=np.float32)
    w_out = np.asarray(w_out, dtype=np.float32)
    b_out = np.asarray(b_out, dtype=np.float32)
    # V-projection bias passes through the softmax average; fold it into the
    # output bias: y += b_v @ w_out + b_out
    b_eff = b_qkv[2 * D:] @ w_out + b_out
    out = np.empty((B, S, D), dtype=np.float32)
    for b in range(B):
        out[b] = (res.results[2 * b]["y"].astype(np.float32)
                  + res.results[2 * b + 1]["y"].astype(np.float32) + b_eff)
    return out


def kernel(x, w_qkv, b_qkv, w_out, b_out):
    global LAST_RESULTS
    in_maps = make_in_maps(x, w_qkv, b_qkv, w_out, b_out)
    nc = _get_nc()
    res = run_bass_kernel_spmd(nc, in_maps, core_ids=list(range(N_CORES)))
    LAST_RESULTS = res
    return finish_output(res, x, w_qkv, b_qkv, w_out, b_out)



# revision 54
# speedup vs baseline: 1.1512x; 1.1512x over previous
"""Multi-head attention (B=4, S=2048, D=768, H=12) on 8 TRN2 NeuronCores.

Sharding: core i -> batch i//2, heads 6*(i%2) .. 6*(i%2)+6 (48 (b,h) pairs,
6 per core). Each core computes q^T/k^T in [d, s] layout, V in natural
[s, d] layout (bf16, with an appended ones-column so the softmax denominator
falls out of the attnV matmul), transposed scores S^T[k, q], exp on the
scalar engine (bf16 out), then the partial output projection over its 384
attention-output channels. The two cores sharing a batch have their partial
projections summed host-side, which stands in for the tensor-parallel
all-reduce.

Performance structure:
- Matmuls in f32r (TF32-like, 1 cyc/row at N>=256); paired score matmuls in
  64-row groups; attention inner loop software-pipelined (attnV trails the
  exp stream by TRAIL k-blocks; divisions of head pair j run inside pair
  j+1's loop).
- Attention processed per (head-pair, 512-wide q-quarter) so every psum
  tile is one bank: 6 rotating score tiles + 2 attnV accumulators. HW
  cross-engine dependency latency is far above the cost model's, so deep
  score rotation (3 k-blocks of slack on the scores->exp handoff) is worth
  more than big tiles (HW-swept: 6/2 = 434us vs 4/4 = 465us).
- exp split across engines: each k-block's TWO head-scores exp in ONE
  instruction (adjacent expS slots; halves the handoff count); odd
  k-blocks >= 3 on the DVE via Schraudolph int16-bitcast (A_SCH*s + B_SCH
  -> bf16 bits), the rest on ACT (table exp), keeping ACT under the PE's
  per-k-block budget at ~+0.8% attention-output error (gate is 2e-2).
- Output projection transposed (y^T[o,s] = wout^T @ attnT) with bf16
  wout stationary and bf16 attnT moving in all-512 chunks: the f32r
  outproj path (256-wide moving chunks / f32r operands) ran ~3x slower
  than modeled on HW (-58us measured); host un-transposes while summing
  the partials.
- Softmax denominator via an appended ones-column in V; normalize chain
  reciprocal (DVE) -> partition_broadcast (gpsimd) -> multiply (DVE), no
  PE matmul, issued two k-blocks early through the carry pipeline.
- x^T / w_qkv / w_v inputs in bf16: halves per-rep input DMA (9.8 -> 4.9 MB)
  and SBUF footprint; V/expS tiles bf16. Input DMAs split across both
  HWDGE queues (SP+ACT), first chunks small so the first matmul starts
  ~2us earlier; exp-table warm + bqk load issued after the critical DMAs.
- V-projection bias folded to the host (softmax rows sum to 1):
  y += b_v @ w_out + b_out in finish_output().
- Phase-B psum pool at bufs=8; phase D has its own 4-buffer psum pool and
  runs sb 8-15 first so the last pairs' div chains hide behind it; output
  staging + y DMA in bf16 (partials upcast to fp32 on the host).
- Timing methodology: wall-clock through the axon tunnel is dominated by
  per-call payload (~30MB/s, +-0.3s jitter), so exec time is measured with
  _build(hw_reps=K): a sequencer For_i loop around the body (constant NEFF
  size); (t[K=4001] - t[K=1]) / 4000 resolves per-rep time to a few us
  (bench3.py).
"""

import os

import numpy as np

import concourse.bass as bass
from concourse import bacc
import concourse.mybir as mybir
import concourse.tile as tile
from concourse.bass_utils import run_bass_kernel_spmd

F32 = mybir.dt.float32
F32R = mybir.dt.float32r
BF16 = mybir.dt.bfloat16
I16 = mybir.dt.int16
AF = mybir.ActivationFunctionType
ALU = mybir.AluOpType

B, S, D = 4, 2048, 768
H, HD = 12, 64
HPC = 6            # heads per core
GC = HPC * HD      # 384 channels per core
N_CORES = 8
SCALE = 1.0 / np.sqrt(np.float32(H))   # NOTE: reference scales by 1/sqrt(H)

# Schraudolph constants: bits = A_SCH * s_raw + B_SCH, int16 -> bitcast bf16
A_SCH = float(128.0 * np.log2(np.e) * SCALE)
B_SCH = float(128.0 * 127.0 - 7.5)

SPLIT_ATTNV = False
SPLIT_PROJ = False
SPLIT_OUTPROJ = False
# k-blocks whose (merged two-head) exp runs on the DVE (Schraudolph)
# instead of the scalar engine: unloads ACT (the phase-C co-critical
# engine) at ~1.2% extra attention-output error (validated in numpy; gate
# is 2e-2). Odd k-blocks >= 3: kb 0-2 stay on ACT so the DVE is clear of
# the div-carry burst (reciprocal + norm) at the start of each pair.
DVE_KBS = frozenset({3, 5, 7, 9, 11, 13, 15})
if os.environ.get("KNOB_DVE_KBS") is not None:
    _v = os.environ["KNOB_DVE_KBS"]
    DVE_KBS = frozenset(int(x) for x in _v.split(",") if x != "")
SKIP = frozenset()
# psum split: 3 two-bank score tiles (one per k-block holding BOTH heads,
# 3-kb rotation slack on the scores->exp handoff, the dominant HW stall)
# + 2 single-bank attnV accumulators. Predecessor sweep (per-head tiles):
# 4/4=465us, 5/3=441us, 6/2=434us on the For_i paired bench.
PSS_BUFS = int(os.environ.get("KNOB_PSS_BUFS", "3"))
PSO_BUFS = int(os.environ.get("KNOB_PSO_BUFS", "2"))

_NC_CACHE = None
LAST_RESULTS = None


def _build(reps=1, hw_reps=1):
    """reps: python-unrolled repetitions (NEFF grows per rep).
    hw_reps: sequencer-level For_i loop around the body (constant NEFF size;
    used for tunnel-immune on-device timing via large iteration counts)."""
    nc = bacc.Bacc("TRN2", target_bir_lowering=False, debug=False,
                   num_devices=N_CORES)
    xt = nc.dram_tensor("xt", (D, S), BF16, kind="ExternalInput")
    wqk = nc.dram_tensor("wqk", (D, 2 * GC), BF16, kind="ExternalInput")
    wv = nc.dram_tensor("wv", (D, GC), BF16, kind="ExternalInput")
    bqk = nc.dram_tensor("bqk", (2 * GC,), F32, kind="ExternalInput")
    wout = nc.dram_tensor("wout", (GC, D), BF16, kind="ExternalInput")
    # y stored transposed [D, S]; the host sums the two per-batch partials
    # anyway, so it un-transposes for free in finish_output
    y = nc.dram_tensor("y", (D, S), BF16, kind="ExternalOutput")

    NSB = S // 128        # 16 s-blocks
    NDS = D // 128        # 6 d-subtiles
    NOB = 2 * GC // 128   # 6 q+k output blocks
    NCS = GC // 128       # 3 c-subtiles for out-proj

    with tile.TileContext(nc) as tc:
        with (
            tc.tile_pool(name="const", bufs=1) as cpool,
            tc.tile_pool(name="bigqv", bufs=1) as bigqv,
            tc.tile_pool(name="bigd", bufs=1) as bigd,
        ):
            ones_sb = cpool.tile([1, 128], F32)
            nc.gpsimd.memset(ones_sb[:], 1.0)
            ones_r = cpool.tile([1, 128], F32R)
            nc.vector.tensor_copy(ones_r[:], ones_sb[:])
            # exp ACT table preload + bqk load are emitted inside rep_body
            # AFTER the phase-B input-DMA dispatches: both sit on queues
            # (ACT/SP) whose first dma_starts gate the very first matmul
            warm = cpool.tile([1, 8], F32)
            bqk_sb = cpool.tile([128, NOB], F32)
            wout_sb = cpool.tile([128, NCS, D], BF16)

            def warm_and_bqk():
                nc.sync.dma_start(
                    bqk_sb[:], bqk.ap().rearrange("(ob p) -> p ob", p=128))
                nc.scalar.activation(warm[:], ones_sb[:, :8], AF.Exp)

            qkT = bigqv.tile([128, NOB, S], F32R)     # blocks 0-2 q^T, 3-5 k^T
            V_sb = bigqv.tile([128, NSB, HPC * (HD + 1)], BF16)  # V + ones col
            attnT = bigd.tile([128, NCS, S], BF16)    # attention out, [c, s]

            if "div" in SKIP:     # ablation builds: attnT must have a writer
                nc.vector.memset(attnT[:], 0.0)

            V_view = V_sb[:].rearrange("p b (h e) -> p b h e", e=HD + 1)
            ones_col = cpool.tile([128, 1], BF16)
            nc.gpsimd.memset(ones_col[:], 1.0)
            nc.vector.tensor_copy(
                V_view[:, :, :, HD], ones_col[:, :, None].to_broadcast([128, NSB, HPC])
            )

            xt_src = xt.ap().rearrange("(ds p) s -> p ds s", p=128)
            wqk_src = wqk.ap().rearrange("(ds p) o -> p ds o", p=128)

            rep_ctx = tc.tile_pool(name="xtp", bufs=2)
            xtp = rep_ctx.__enter__()

            def rep_body(load_wout):
                # ---- Phase B: projections ----
                OB_ORDER = (0, 3, 1, 4, 2, 5)
                with (
                    tc.tile_pool(name="psb", bufs=8, space="PSUM") as psb,
                ):
                    xt_sb = xtp.tile([128, NDS, S], BF16, tag="xt",
                                     name="xt_sb")
                    wqk_sb = xtp.tile([128, NDS, 2 * GC], BF16, tag="wq",
                                      name="wqk_sb")
                    wv_sb = xtp.tile([128, NDS, GC], BF16, tag="wv",
                                     name="wv_sb")
                    # input DMAs split across the two HWDGE queues (SP+ACT,
                    # ~600ns descriptor-gen each, serial per queue); xt's
                    # first chunk in ds-halves so the first ob-0 matmuls
                    # (ds-major) start as early as possible
                    nc.scalar.dma_start(
                        wqk_sb[:, :, 0:128], wqk_src[:, :, 0:128],
                    )
                    for d0, d1 in ((0, 3), (3, 6)):
                        nc.sync.dma_start(
                            xt_sb[:, d0:d1, 0:512],
                            xt_src[:, d0:d1, 0:512],
                        )
                    nc.scalar.dma_start(
                        xt_sb[:, :, 512:1024], xt_src[:, :, 512:1024],
                    )
                    nc.scalar.dma_start(
                        wqk_sb[:, :, 3 * 128:4 * 128],
                        wqk_src[:, :, 3 * 128:4 * 128],
                    )
                    for sc, eng in ((2, nc.sync), (3, nc.scalar)):
                        eng.dma_start(
                            xt_sb[:, :, sc * 512:(sc + 1) * 512],
                            xt_src[:, :, sc * 512:(sc + 1) * 512],
                        )
                    if load_wout:
                        warm_and_bqk()
                    for ob in (1, 4, 2, 5):
                        nc.sync.dma_start(
                            wqk_sb[:, :, ob * 128:(ob + 1) * 128],
                            wqk_src[:, :, ob * 128:(ob + 1) * 128],
                        )
                    nc.sync.dma_start(
                        wv_sb[:],
                        wv.ap().rearrange("(ds p) o -> p ds o", p=128),
                    )
                    if load_wout:
                        nc.sync.dma_start(
                            wout_sb[:],
                            wout.ap().rearrange("(cs p) o -> p cs o", p=128),
                        )

                    # q^T / k^T: [o, s] = wqk^T @ x^T
                    halves = ((0, 64), (64, 128)) if SPLIT_PROJ else ((0, 128),)
                    for ob in OB_ORDER:
                        pss4 = [psb.tile([128, 512], F32, tag="ps",
                                         name=f"ps{ob}_{sc}") for sc in range(4)]
                        # ob 0 runs sc-outer: its first 6 matmuls then need
                        # only the first xt chunk, which lands ~3us before
                        # the rest of xt
                        if ob == 0:
                            loop_iter = [(ds, sc) for sc in range(4)
                                         for ds in range(NDS)]
                        else:
                            loop_iter = [(ds, sc) for ds in range(NDS)
                                         for sc in range(4)]
                        for ds, sc in loop_iter if "proj" not in SKIP else ():
                            for hi, (r0, r1) in enumerate(halves):
                                nc.tensor.matmul(
                                    pss4[sc][:],
                                    wqk_sb[r0:r1, ds, ob * 128:(ob + 1) * 128],
                                    xt_sb[r0:r1, ds, sc * 512:(sc + 1) * 512],
                                    start=(ds == 0 and hi == 0),
                                    stop=(ds == NDS - 1
                                          and hi == len(halves) - 1),
                                    skip_group_check=SPLIT_PROJ,
                                )
                        for sc in range(4):
                            nc.vector.tensor_scalar_add(
                                qkT[:, ob, sc * 512:(sc + 1) * 512], pss4[sc][:],
                                bqk_sb[:, ob:ob + 1],
                            )

                    # V natural: [s, o] = x @ wv   (bias folded to host)
                    for sb in range(NSB):
                        ps = psb.tile([128, 512], F32, tag="ps")
                        for ds in range(NDS) if "proj" not in SKIP else ():
                            for hi, (r0, r1) in enumerate(halves):
                                nc.tensor.matmul(
                                    ps[:, :GC],
                                    xt_sb[r0:r1, ds, sb * 128:(sb + 1) * 128],
                                    wv_sb[r0:r1, ds, :],
                                    start=(ds == 0 and hi == 0),
                                    stop=(ds == NDS - 1
                                          and hi == len(halves) - 1),
                                    skip_group_check=True,
                                )
                        nc.vector.tensor_copy(V_view[:, sb, :, 0:HD], ps[:, :GC])

                # ---- Phase C: attention per (head-pair, q-quarter) ----
                # q processed in 512-wide quarters so every psum tile is a
                # single bank: pss 4 bufs + pso 4 bufs fills the 8 banks and
                # doubles the rotation slack on every cross-engine handoff
                # (HW dependency latency is far above the cost model's 100ns
                # -- measured via SKIP ablations on the For_i bench).
                with (
                    tc.tile_pool(name="bigc", bufs=1) as bigc,
                    tc.tile_pool(name="cw", bufs=1) as cw,
                    tc.tile_pool(name="pss", bufs=PSS_BUFS, space="PSUM") as pss,
                    tc.tile_pool(name="pso", bufs=PSO_BUFS, space="PSUM") as pso,
                ):
                    QQ = 512
                    NQQ = S // QQ  # 4 q-quarters
                    NBUF = 12      # rotating S^T exp slots (2 per k-block)
                    expS = bigc.tile([128, NBUF, QQ], BF16)

                    def slot(kb, hi):
                        return (2 * kb + hi) % NBUF

                    def make_div(ps_o, h, qq):
                        # normalize out'[d, q] by Z[q] (ones-column row).
                        # reciprocal [1,QQ] (DVE, from psum), partition-
                        # broadcast (gpsimd), multiply (DVE): no PE matmul,
                        # no shared-psum-pool collision with the score tiles.
                        # Split into two stages so both heads' reciprocals and
                        # broadcasts issue before either multiply (in-order
                        # engine queues), letting ps_o free as early as
                        # possible for the next pair's attnV.
                        if "div" in SKIP:
                            return lambda: None, lambda: None
                        base = (h % 2) * 64
                        qob = h // 2
                        rz = cw.tile([1, QQ], F32, tag="rz", bufs=2, name="rz")
                        rzb_sb = cw.tile([64, QQ], F32, tag="rzb", bufs=2,
                                         name="rzb_sb")

                        def recip_bcast():
                            with nc.allow_low_precision(reason="f32r softmax denom"):
                                nc.vector.reciprocal(rz[:], ps_o[HD:HD + 1, :])
                            nc.gpsimd.partition_broadcast(rzb_sb[:], rz[:])

                        def norm():
                            nc.vector.tensor_mul(
                                attnT[base:base + 64, qob, qq * QQ:(qq + 1) * QQ],
                                ps_o[0:HD, :], rzb_sb[:],
                            )
                        return recip_bcast, norm

                    vhalves = ((0, 64), (64, 128)) if SPLIT_ATTNV else ((0, 128),)

                    def attn_v(ps_o, h, kb, start, stop):
                        if "attnv" in SKIP:
                            return
                        sl = slot(kb, h % 2)
                        for hi, (r0, r1) in enumerate(vhalves):
                            nc.tensor.matmul(
                                ps_o[:, :],
                                V_sb[r0:r1, kb,
                                     h * (HD + 1):(h + 1) * (HD + 1)],
                                expS[r0:r1, sl, :],
                                start=(start and hi == 0),
                                stop=(stop and hi == len(vhalves) - 1),
                                skip_group_check=True,
                            )

                    def do_exp(ps_s, kb):
                        # ONE instruction covers both heads' scores: the two
                        # expS slots for (kb,0)/(kb,1) are adjacent (slot of
                        # (kb,0) is always even), halving the exp instruction
                        # and cross-engine handoff count per k-block
                        if "exp" in SKIP:
                            return
                        sl = slot(kb, 0)
                        if kb in DVE_KBS:
                            nc.vector.tensor_scalar(
                                expS[:, sl:sl + 2, :].bitcast(I16), ps_s[:],
                                A_SCH, B_SCH, ALU.mult, ALU.add,
                            )
                        else:
                            nc.scalar.activation(
                                expS[:, sl:sl + 2, :], ps_s[:], AF.Exp,
                                scale=float(SCALE),
                            )

                    TRAIL = 4
                    carry = []   # closures from the previous (pair, qq)
                    # qq 2,3 first: phase D then starts with sb 8-15
                    # (ready long before), hiding the last pairs' div chains
                    for qq in (2, 3, 0, 1):
                        for hp in range(HPC // 2):
                            hA, hB = 2 * hp, 2 * hp + 1
                            qob = hp
                            kob = NCS + hp
                            ps_oA = pso.tile([HD + 1, QQ], F32, tag="o", name="ps_oA")
                            ps_oB = pso.tile([HD + 1, QQ], F32, tag="o", name="ps_oB")
                            for kb in range(NSB):
                                ps_s = pss.tile([128, 2, QQ], F32, tag="s",
                                                name="ps_s")
                                for hi, base in ((0, 0), (1, 64)) \
                                        if "scores" not in SKIP else ():
                                    nc.tensor.matmul(
                                        ps_s[:, hi, :],
                                        qkT[base:base + 64, kob,
                                            kb * 128:(kb + 1) * 128],
                                        qkT[base:base + 64, qob,
                                            qq * QQ:(qq + 1) * QQ],
                                        start=True, stop=True,
                                    )
                                do_exp(ps_s, kb)
                                if kb < len(carry):
                                    carry[kb]()
                                if kb >= TRAIL:
                                    pk = kb - TRAIL
                                    attn_v(ps_oA, hA, pk, start=(pk == 0), stop=False)
                                    attn_v(ps_oB, hB, pk, start=(pk == 0), stop=False)
                            for pk in range(NSB - TRAIL, NSB - 1):
                                attn_v(ps_oA, hA, pk, start=False, stop=False)
                                attn_v(ps_oB, hB, pk, start=False, stop=False)
                            rbA, normA = make_div(ps_oA, hA, qq)
                            rbB, normB = make_div(ps_oB, hB, qq)
                            carry = [
                                lambda a=ps_oA, b=ps_oB, h1=hA, h2=hB, \
                                        rA=rbA, rB=rbB: (
                                    attn_v(a, h1, NSB - 1, start=False, stop=True),
                                    attn_v(b, h2, NSB - 1, start=False, stop=True),
                                    rA(), rB(),
                                ),
                                lambda nA=normA, nB=normB: (nA(), nB()),
                            ]
                    for f in carry:
                        f()

                # ---- Phase D: output projection (partial, 384 c) ----
                # transposed: y^T[o, s] = wout^T @ attnT with wout (bf16) as
                # the stationary operand and attnT (f32r) moving in all-512
                # chunks -- no 256-wide f32r moving chunks (rate-sensitive on
                # HW) and no f32r stationary loads. Own psum pool (phase C's
                # pools are closed); q-chunks 2,3 first (normalized pairs
                # ago) so the last pairs' div chains hide behind them.
                with (
                    tc.tile_pool(name="psd", bufs=6, space="PSUM") as psd,
                    tc.tile_pool(name="cwd", bufs=1) as cwd,
                ):
                    y_dst = y.ap().rearrange("(ob p) s -> p ob s", p=128)
                    for sc in (2, 3, 0, 1):
                        for ob in range(D // 128):
                            ps_f = psd.tile([128, QQ], F32, tag="f",
                                            name="ps_f")
                            for cs in range(NCS) if "outproj" not in SKIP else ():
                                nc.tensor.matmul(
                                    ps_f[:],
                                    wout_sb[:, cs, ob * 128:(ob + 1) * 128],
                                    attnT[:, cs, sc * QQ:(sc + 1) * QQ],
                                    start=(cs == 0), stop=(cs == NCS - 1),
                                )
                            if "outproj" not in SKIP:
                                ostage = cwd.tile([128, QQ], BF16,
                                                  tag="ostage", bufs=4,
                                                  name="ostage")
                                # ACT is idle during phase D; keep the
                                # psum->sbuf staging off the DVE
                                nc.scalar.activation(ostage[:], ps_f[:],
                                                     AF.Copy)
                                nc.sync.dma_start(
                                    y_dst[:, ob, sc * QQ:(sc + 1) * QQ],
                                    ostage[:])

            if hw_reps > 1:
                warm_and_bqk()
                nc.sync.dma_start(
                    wout_sb[:],
                    wout.ap().rearrange("(cs p) o -> p cs o", p=128),
                )
                with tc.For_i(0, hw_reps):
                    rep_body(load_wout=False)
            else:
                for _rep in range(reps):
                    rep_body(load_wout=(_rep == 0))
            rep_ctx.__exit__(None, None, None)

    nc.compile()
    return nc


def _get_nc():
    global _NC_CACHE
    if _NC_CACHE is None:
        _NC_CACHE = _build()
    return _NC_CACHE


def make_in_maps(x, w_qkv, b_qkv, w_out, b_out):
    x = np.asarray(x, dtype=np.float32)
    w_qkv = np.asarray(w_qkv, dtype=np.float32)
    b_qkv = np.asarray(b_qkv, dtype=np.float32)
    w_out = np.asarray(w_out, dtype=np.float32)

    in_maps = []
    for i in range(N_CORES):
        b = i // 2
        c0 = (i % 2) * GC
        q_sl = slice(c0, c0 + GC)
        k_sl = slice(D + c0, D + c0 + GC)
        v_sl = slice(2 * D + c0, 2 * D + c0 + GC)
        import ml_dtypes
        bf = ml_dtypes.bfloat16
        in_maps.append({
            "xt": np.ascontiguousarray(x[b].T.astype(bf)),
            "wqk": np.ascontiguousarray(np.concatenate(
                [w_qkv[:, q_sl], w_qkv[:, k_sl]], axis=1).astype(bf)),
            "wv": np.ascontiguousarray(w_qkv[:, v_sl].astype(bf)),
            "bqk": np.ascontiguousarray(
                np.concatenate([b_qkv[q_sl], b_qkv[k_sl]])),
            "wout": np.ascontiguousarray(w_out[c0:c0 + GC, :].astype(bf)),
        })
    return in_maps


def finish_output(res, x, w_qkv, b_qkv, w_out, b_out):
    b_qkv = np.asarray(b_qkv, dtype=np.float32)
    w_out = np.asarray(w_out, dtype=np.float32)
    b_out = np.asarray(b_out, dtype=np.float32)
    # V-projection bias passes through the softmax average; fold it into the
    # output bias: y += b_v @ w_out + b_out
    b_eff = b_qkv[2 * D:] @ w_out + b_out
    out = np.empty((B, S, D), dtype=np.float32)
    for b in range(B):
        # y comes back transposed [D, S]; un-transpose while summing the
        # two head-group partials
        out[b] = (res.results[2 * b]["y"].astype(np.float32)
                  + res.results[2 * b + 1]["y"].astype(np.float32)).T + b_eff
    return out


def kernel(x, w_qkv, b_qkv, w_out, b_out):
    global LAST_RESULTS
    in_maps = make_in_maps(x, w_qkv, b_qkv, w_out, b_out)
    nc = _get_nc()
    res = run_bass_kernel_spmd(nc, in_maps, core_ids=list(range(N_CORES)))
    LAST_RESULTS = res
    return finish_output(res, x, w_qkv, b_qkv, w_out, b_out)



# revision 56
# speedup vs baseline: 1.1930x; 1.0364x over previous
"""Multi-head attention (B=4, S=2048, D=768, H=12) on 8 TRN2 NeuronCores.

Sharding: core i -> batch i//2, heads 6*(i%2) .. 6*(i%2)+6 (48 (b,h) pairs,
6 per core). Each core computes q^T/k^T in [d, s] layout, V in natural
[s, d] layout (bf16, with an appended ones-column so the softmax denominator
falls out of the attnV matmul), transposed scores S^T[k, q], exp on the
scalar engine (bf16 out), then the partial output projection over its 384
attention-output channels. The two cores sharing a batch have their partial
projections summed host-side, which stands in for the tensor-parallel
all-reduce.

Performance structure:
- ALL matmul operands bf16 (xt/wqk/wv/qkT/V/expS/attnT/wout): the HW f32r
  matmul path measured far below its cost-model rate (qkT bf16 alone was
  -13us, the outproj f32r->bf16 transpose -58us). Paired score matmuls in
  64-row groups; attention inner loop software-pipelined (attnV trails the
  exp stream by TRAIL k-blocks; divisions of head pair j run inside pair
  j+1's loop).
- Attention processed per (head-pair, 512-wide q-quarter) so every psum
  tile is one bank: 6 rotating score tiles + 2 attnV accumulators. HW
  cross-engine dependency latency is far above the cost model's, so deep
  score rotation (3 k-blocks of slack on the scores->exp handoff) is worth
  more than big tiles (HW-swept: 6/2 = 434us vs 4/4 = 465us).
- exp split across engines: each k-block's TWO head-scores exp in ONE
  instruction (adjacent expS slots; halves the handoff count); odd
  k-blocks >= 3 on the DVE via Schraudolph int16-bitcast (A_SCH*s + B_SCH
  -> bf16 bits), the rest on ACT (table exp), keeping ACT under the PE's
  per-k-block budget at ~+0.8% attention-output error (gate is 2e-2).
- Output projection transposed (y^T[o,s] = wout^T @ attnT) with bf16
  wout stationary and bf16 attnT moving in all-512 chunks: the f32r
  outproj path (256-wide moving chunks / f32r operands) ran ~3x slower
  than modeled on HW (-58us measured); host un-transposes while summing
  the partials.
- Softmax denominator via an appended ones-column in V; normalize chain
  reciprocal (DVE) -> partition_broadcast (gpsimd) -> multiply (DVE), no
  PE matmul, issued two k-blocks early through the carry pipeline.
- x^T / w_qkv / w_v inputs in bf16: halves per-rep input DMA (9.8 -> 4.9 MB)
  and SBUF footprint; V/expS tiles bf16. Input DMAs split across both
  HWDGE queues (SP+ACT), first chunks small so the first matmul starts
  ~2us earlier; exp-table warm + bqk load issued after the critical DMAs.
- V-projection bias folded to the host (softmax rows sum to 1):
  y += b_v @ w_out + b_out in finish_output().
- Phase-B psum pool at bufs=8; phase D has its own 4-buffer psum pool and
  runs sb 8-15 first so the last pairs' div chains hide behind it; output
  staging + y DMA in bf16 (partials upcast to fp32 on the host).
- Timing methodology: wall-clock through the axon tunnel is dominated by
  per-call payload (~30MB/s, +-0.3s jitter), so exec time is measured with
  _build(hw_reps=K): a sequencer For_i loop around the body (constant NEFF
  size); (t[K=4001] - t[K=1]) / 4000 resolves per-rep time to a few us
  (bench3.py).
"""

import os

import numpy as np

import concourse.bass as bass
from concourse import bacc
import concourse.mybir as mybir
import concourse.tile as tile
from concourse.bass_utils import run_bass_kernel_spmd

F32 = mybir.dt.float32
F32R = mybir.dt.float32r
BF16 = mybir.dt.bfloat16
I16 = mybir.dt.int16
AF = mybir.ActivationFunctionType
ALU = mybir.AluOpType

B, S, D = 4, 2048, 768
H, HD = 12, 64
HPC = 6            # heads per core
GC = HPC * HD      # 384 channels per core
N_CORES = 8
SCALE = 1.0 / np.sqrt(np.float32(H))   # NOTE: reference scales by 1/sqrt(H)

# Schraudolph constants: bits = A_SCH * s_raw + B_SCH, int16 -> bitcast bf16
A_SCH = float(128.0 * np.log2(np.e) * SCALE)
B_SCH = float(128.0 * 127.0 - 7.5)

SPLIT_ATTNV = False
SPLIT_PROJ = False
SPLIT_OUTPROJ = False
# k-blocks whose (merged two-head) exp runs on the DVE (Schraudolph)
# instead of the scalar engine: unloads ACT (the phase-C co-critical
# engine) at ~1.2% extra attention-output error (validated in numpy; gate
# is 2e-2). Odd k-blocks >= 3: kb 0-2 stay on ACT so the DVE is clear of
# the div-carry burst (reciprocal + norm) at the start of each pair.
DVE_KBS = frozenset({3, 5, 7, 9, 11, 13, 15})
if os.environ.get("KNOB_DVE_KBS") is not None:
    _v = os.environ["KNOB_DVE_KBS"]
    DVE_KBS = frozenset(int(x) for x in _v.split(",") if x != "")
SKIP = frozenset()
# psum split: 3 two-bank score tiles (one per k-block holding BOTH heads,
# 3-kb rotation slack on the scores->exp handoff, the dominant HW stall)
# + 2 single-bank attnV accumulators. Predecessor sweep (per-head tiles):
# 4/4=465us, 5/3=441us, 6/2=434us on the For_i paired bench.
PSS_BUFS = int(os.environ.get("KNOB_PSS_BUFS", "3"))
PSO_BUFS = int(os.environ.get("KNOB_PSO_BUFS", "2"))

_NC_CACHE = None
LAST_RESULTS = None


def _build(reps=1, hw_reps=1):
    """reps: python-unrolled repetitions (NEFF grows per rep).
    hw_reps: sequencer-level For_i loop around the body (constant NEFF size;
    used for tunnel-immune on-device timing via large iteration counts)."""
    nc = bacc.Bacc("TRN2", target_bir_lowering=False, debug=False,
                   num_devices=N_CORES)
    xt = nc.dram_tensor("xt", (D, S), BF16, kind="ExternalInput")
    wqk = nc.dram_tensor("wqk", (D, 2 * GC), BF16, kind="ExternalInput")
    wv = nc.dram_tensor("wv", (D, GC), BF16, kind="ExternalInput")
    bqk = nc.dram_tensor("bqk", (2 * GC,), F32, kind="ExternalInput")
    wout = nc.dram_tensor("wout", (GC, D), BF16, kind="ExternalInput")
    # y stored transposed [D, S]; the host sums the two per-batch partials
    # anyway, so it un-transposes for free in finish_output
    y = nc.dram_tensor("y", (D, S), BF16, kind="ExternalOutput")

    NSB = S // 128        # 16 s-blocks
    NDS = D // 128        # 6 d-subtiles
    NOB = 2 * GC // 128   # 6 q+k output blocks
    NCS = GC // 128       # 3 c-subtiles for out-proj

    with tile.TileContext(nc) as tc:
        with (
            tc.tile_pool(name="const", bufs=1) as cpool,
            tc.tile_pool(name="bigqv", bufs=1) as bigqv,
            tc.tile_pool(name="bigd", bufs=1) as bigd,
        ):
            ones_sb = cpool.tile([1, 128], F32)
            nc.gpsimd.memset(ones_sb[:], 1.0)
            ones_r = cpool.tile([1, 128], F32R)
            nc.vector.tensor_copy(ones_r[:], ones_sb[:])
            # exp ACT table preload + bqk load are emitted inside rep_body
            # AFTER the phase-B input-DMA dispatches: both sit on queues
            # (ACT/SP) whose first dma_starts gate the very first matmul
            warm = cpool.tile([1, 8], F32)
            bqk_sb = cpool.tile([128, NOB], F32)
            wout_sb = cpool.tile([128, NCS, D], BF16)

            def warm_and_bqk():
                nc.sync.dma_start(
                    bqk_sb[:], bqk.ap().rearrange("(ob p) -> p ob", p=128))
                nc.scalar.activation(warm[:], ones_sb[:, :8], AF.Exp)

            # bf16 q^T/k^T: avoids the HW f32r matmul-path penalty found on
            # the output projection, and halves the tile (48 -> 24 KB/part)
            qkT = bigqv.tile([128, NOB, S], BF16)     # blocks 0-2 q^T, 3-5 k^T
            V_sb = bigqv.tile([128, NSB, HPC * (HD + 1)], BF16)  # V + ones col
            attnT = bigd.tile([128, NCS, S], BF16)    # attention out, [c, s]

            if "div" in SKIP:     # ablation builds: attnT must have a writer
                nc.vector.memset(attnT[:], 0.0)

            V_view = V_sb[:].rearrange("p b (h e) -> p b h e", e=HD + 1)
            ones_col = cpool.tile([128, 1], BF16)
            nc.gpsimd.memset(ones_col[:], 1.0)
            nc.vector.tensor_copy(
                V_view[:, :, :, HD], ones_col[:, :, None].to_broadcast([128, NSB, HPC])
            )

            xt_src = xt.ap().rearrange("(ds p) s -> p ds s", p=128)
            wqk_src = wqk.ap().rearrange("(ds p) o -> p ds o", p=128)

            rep_ctx = tc.tile_pool(name="xtp", bufs=2)
            xtp = rep_ctx.__enter__()

            def rep_body(load_wout):
                # ---- Phase B: projections ----
                OB_ORDER = (0, 3, 1, 4, 2, 5)
                with (
                    tc.tile_pool(name="psb", bufs=8, space="PSUM") as psb,
                ):
                    xt_sb = xtp.tile([128, NDS, S], BF16, tag="xt",
                                     name="xt_sb")
                    wqk_sb = xtp.tile([128, NDS, 2 * GC], BF16, tag="wq",
                                      name="wqk_sb")
                    wv_sb = xtp.tile([128, NDS, GC], BF16, tag="wv",
                                     name="wv_sb")
                    # input DMAs split across the two HWDGE queues (SP+ACT,
                    # ~600ns descriptor-gen each, serial per queue); xt's
                    # first chunk in ds-halves so the first ob-0 matmuls
                    # (ds-major) start as early as possible
                    nc.scalar.dma_start(
                        wqk_sb[:, :, 0:128], wqk_src[:, :, 0:128],
                    )
                    for d0, d1 in ((0, 3), (3, 6)):
                        nc.sync.dma_start(
                            xt_sb[:, d0:d1, 0:512],
                            xt_src[:, d0:d1, 0:512],
                        )
                    nc.scalar.dma_start(
                        xt_sb[:, :, 512:1024], xt_src[:, :, 512:1024],
                    )
                    nc.scalar.dma_start(
                        wqk_sb[:, :, 3 * 128:4 * 128],
                        wqk_src[:, :, 3 * 128:4 * 128],
                    )
                    for sc, eng in ((2, nc.sync), (3, nc.scalar)):
                        eng.dma_start(
                            xt_sb[:, :, sc * 512:(sc + 1) * 512],
                            xt_src[:, :, sc * 512:(sc + 1) * 512],
                        )
                    if load_wout:
                        warm_and_bqk()
                    for ob in (1, 4, 2, 5):
                        nc.sync.dma_start(
                            wqk_sb[:, :, ob * 128:(ob + 1) * 128],
                            wqk_src[:, :, ob * 128:(ob + 1) * 128],
                        )
                    nc.sync.dma_start(
                        wv_sb[:],
                        wv.ap().rearrange("(ds p) o -> p ds o", p=128),
                    )
                    if load_wout:
                        nc.sync.dma_start(
                            wout_sb[:],
                            wout.ap().rearrange("(cs p) o -> p cs o", p=128),
                        )

                    # q^T / k^T: [o, s] = wqk^T @ x^T
                    halves = ((0, 64), (64, 128)) if SPLIT_PROJ else ((0, 128),)
                    for ob in OB_ORDER:
                        pss4 = [psb.tile([128, 512], F32, tag="ps",
                                         name=f"ps{ob}_{sc}") for sc in range(4)]
                        # ob 0 runs sc-outer: its first 6 matmuls then need
                        # only the first xt chunk, which lands ~3us before
                        # the rest of xt
                        if ob == 0:
                            loop_iter = [(ds, sc) for sc in range(4)
                                         for ds in range(NDS)]
                        else:
                            loop_iter = [(ds, sc) for ds in range(NDS)
                                         for sc in range(4)]
                        for ds, sc in loop_iter if "proj" not in SKIP else ():
                            for hi, (r0, r1) in enumerate(halves):
                                nc.tensor.matmul(
                                    pss4[sc][:],
                                    wqk_sb[r0:r1, ds, ob * 128:(ob + 1) * 128],
                                    xt_sb[r0:r1, ds, sc * 512:(sc + 1) * 512],
                                    start=(ds == 0 and hi == 0),
                                    stop=(ds == NDS - 1
                                          and hi == len(halves) - 1),
                                    skip_group_check=SPLIT_PROJ,
                                )
                        for sc in range(4):
                            nc.vector.tensor_scalar_add(
                                qkT[:, ob, sc * 512:(sc + 1) * 512], pss4[sc][:],
                                bqk_sb[:, ob:ob + 1],
                            )

                    # V natural: [s, o] = x @ wv   (bias folded to host)
                    for sb in range(NSB):
                        ps = psb.tile([128, 512], F32, tag="ps")
                        for ds in range(NDS) if "proj" not in SKIP else ():
                            for hi, (r0, r1) in enumerate(halves):
                                nc.tensor.matmul(
                                    ps[:, :GC],
                                    xt_sb[r0:r1, ds, sb * 128:(sb + 1) * 128],
                                    wv_sb[r0:r1, ds, :],
                                    start=(ds == 0 and hi == 0),
                                    stop=(ds == NDS - 1
                                          and hi == len(halves) - 1),
                                    skip_group_check=True,
                                )
                        nc.vector.tensor_copy(V_view[:, sb, :, 0:HD], ps[:, :GC])

                # ---- Phase C: attention per (head-pair, q-quarter) ----
                # q processed in 512-wide quarters so every psum tile is a
                # single bank: pss 4 bufs + pso 4 bufs fills the 8 banks and
                # doubles the rotation slack on every cross-engine handoff
                # (HW dependency latency is far above the cost model's 100ns
                # -- measured via SKIP ablations on the For_i bench).
                with (
                    tc.tile_pool(name="bigc", bufs=1) as bigc,
                    tc.tile_pool(name="cw", bufs=1) as cw,
                    tc.tile_pool(name="pss", bufs=PSS_BUFS, space="PSUM") as pss,
                    tc.tile_pool(name="pso", bufs=PSO_BUFS, space="PSUM") as pso,
                ):
                    QQ = 512
                    NQQ = S // QQ  # 4 q-quarters
                    NBUF = 12      # rotating S^T exp slots (2 per k-block)
                    expS = bigc.tile([128, NBUF, QQ], BF16)

                    def slot(kb, hi):
                        return (2 * kb + hi) % NBUF

                    def make_div(ps_o, h, qq):
                        # normalize out'[d, q] by Z[q] (ones-column row).
                        # reciprocal [1,QQ] (DVE, from psum), partition-
                        # broadcast (gpsimd), multiply (DVE): no PE matmul,
                        # no shared-psum-pool collision with the score tiles.
                        # Split into two stages so both heads' reciprocals and
                        # broadcasts issue before either multiply (in-order
                        # engine queues), letting ps_o free as early as
                        # possible for the next pair's attnV.
                        if "div" in SKIP:
                            return lambda: None, lambda: None
                        base = (h % 2) * 64
                        qob = h // 2
                        rz = cw.tile([1, QQ], F32, tag="rz", bufs=2, name="rz")
                        rzb_sb = cw.tile([64, QQ], F32, tag="rzb", bufs=2,
                                         name="rzb_sb")

                        def recip_bcast():
                            with nc.allow_low_precision(reason="f32r softmax denom"):
                                nc.vector.reciprocal(rz[:], ps_o[HD:HD + 1, :])
                            nc.gpsimd.partition_broadcast(rzb_sb[:], rz[:])

                        def norm():
                            nc.vector.tensor_mul(
                                attnT[base:base + 64, qob, qq * QQ:(qq + 1) * QQ],
                                ps_o[0:HD, :], rzb_sb[:],
                            )
                        return recip_bcast, norm

                    vhalves = ((0, 64), (64, 128)) if SPLIT_ATTNV else ((0, 128),)

                    def attn_v(ps_o, h, kb, start, stop):
                        if "attnv" in SKIP:
                            return
                        sl = slot(kb, h % 2)
                        for hi, (r0, r1) in enumerate(vhalves):
                            nc.tensor.matmul(
                                ps_o[:, :],
                                V_sb[r0:r1, kb,
                                     h * (HD + 1):(h + 1) * (HD + 1)],
                                expS[r0:r1, sl, :],
                                start=(start and hi == 0),
                                stop=(stop and hi == len(vhalves) - 1),
                                skip_group_check=True,
                            )

                    def do_exp(ps_s, kb):
                        # ONE instruction covers both heads' scores: the two
                        # expS slots for (kb,0)/(kb,1) are adjacent (slot of
                        # (kb,0) is always even), halving the exp instruction
                        # and cross-engine handoff count per k-block
                        if "exp" in SKIP:
                            return
                        sl = slot(kb, 0)
                        if kb in DVE_KBS:
                            nc.vector.tensor_scalar(
                                expS[:, sl:sl + 2, :].bitcast(I16), ps_s[:],
                                A_SCH, B_SCH, ALU.mult, ALU.add,
                            )
                        else:
                            nc.scalar.activation(
                                expS[:, sl:sl + 2, :], ps_s[:], AF.Exp,
                                scale=float(SCALE),
                            )

                    TRAIL = 4
                    carry = []   # closures from the previous (pair, qq)
                    # qq 2,3 first: phase D then starts with sb 8-15
                    # (ready long before), hiding the last pairs' div chains
                    for qq in (2, 3, 0, 1):
                        for hp in range(HPC // 2):
                            hA, hB = 2 * hp, 2 * hp + 1
                            qob = hp
                            kob = NCS + hp
                            ps_oA = pso.tile([HD + 1, QQ], F32, tag="o", name="ps_oA")
                            ps_oB = pso.tile([HD + 1, QQ], F32, tag="o", name="ps_oB")
                            for kb in range(NSB):
                                ps_s = pss.tile([128, 2, QQ], F32, tag="s",
                                                name="ps_s")
                                for hi, base in ((0, 0), (1, 64)) \
                                        if "scores" not in SKIP else ():
                                    nc.tensor.matmul(
                                        ps_s[:, hi, :],
                                        qkT[base:base + 64, kob,
                                            kb * 128:(kb + 1) * 128],
                                        qkT[base:base + 64, qob,
                                            qq * QQ:(qq + 1) * QQ],
                                        start=True, stop=True,
                                    )
                                do_exp(ps_s, kb)
                                if kb < len(carry):
                                    carry[kb]()
                                if kb >= TRAIL:
                                    pk = kb - TRAIL
                                    attn_v(ps_oA, hA, pk, start=(pk == 0), stop=False)
                                    attn_v(ps_oB, hB, pk, start=(pk == 0), stop=False)
                            for pk in range(NSB - TRAIL, NSB - 1):
                                attn_v(ps_oA, hA, pk, start=False, stop=False)
                                attn_v(ps_oB, hB, pk, start=False, stop=False)
                            rbA, normA = make_div(ps_oA, hA, qq)
                            rbB, normB = make_div(ps_oB, hB, qq)
                            carry = [
                                lambda a=ps_oA, b=ps_oB, h1=hA, h2=hB, \
                                        rA=rbA, rB=rbB: (
                                    attn_v(a, h1, NSB - 1, start=False, stop=True),
                                    attn_v(b, h2, NSB - 1, start=False, stop=True),
                                    rA(), rB(),
                                ),
                                lambda nA=normA, nB=normB: (nA(), nB()),
                            ]
                    for f in carry:
                        f()

                # ---- Phase D: output projection (partial, 384 c) ----
                # transposed: y^T[o, s] = wout^T @ attnT with wout (bf16) as
                # the stationary operand and attnT (f32r) moving in all-512
                # chunks -- no 256-wide f32r moving chunks (rate-sensitive on
                # HW) and no f32r stationary loads. Own psum pool (phase C's
                # pools are closed); q-chunks 2,3 first (normalized pairs
                # ago) so the last pairs' div chains hide behind them.
                with (
                    tc.tile_pool(name="psd", bufs=6, space="PSUM") as psd,
                    tc.tile_pool(name="cwd", bufs=1) as cwd,
                ):
                    y_dst = y.ap().rearrange("(ob p) s -> p ob s", p=128)
                    for sc in (2, 3, 0, 1):
                        for ob in range(D // 128):
                            ps_f = psd.tile([128, QQ], F32, tag="f",
                                            name="ps_f")
                            for cs in range(NCS) if "outproj" not in SKIP else ():
                                nc.tensor.matmul(
                                    ps_f[:],
                                    wout_sb[:, cs, ob * 128:(ob + 1) * 128],
                                    attnT[:, cs, sc * QQ:(sc + 1) * QQ],
                                    start=(cs == 0), stop=(cs == NCS - 1),
                                )
                            if "outproj" not in SKIP:
                                ostage = cwd.tile([128, QQ], BF16,
                                                  tag="ostage", bufs=4,
                                                  name="ostage")
                                # ACT is idle during phase D; keep the
                                # psum->sbuf staging off the DVE
                                nc.scalar.activation(ostage[:], ps_f[:],
                                                     AF.Copy)
                                nc.sync.dma_start(
                                    y_dst[:, ob, sc * QQ:(sc + 1) * QQ],
                                    ostage[:])

            if hw_reps > 1:
                warm_and_bqk()
                nc.sync.dma_start(
                    wout_sb[:],
                    wout.ap().rearrange("(cs p) o -> p cs o", p=128),
                )
                with tc.For_i(0, hw_reps):
                    rep_body(load_wout=False)
            else:
                for _rep in range(reps):
                    rep_body(load_wout=(_rep == 0))
            rep_ctx.__exit__(None, None, None)

    nc.compile()
    return nc


def _get_nc():
    global _NC_CACHE
    if _NC_CACHE is None:
        _NC_CACHE = _build()
    return _NC_CACHE


def make_in_maps(x, w_qkv, b_qkv, w_out, b_out):
    x = np.asarray(x, dtype=np.float32)
    w_qkv = np.asarray(w_qkv, dtype=np.float32)
    b_qkv = np.asarray(b_qkv, dtype=np.float32)
    w_out = np.asarray(w_out, dtype=np.float32)

    in_maps = []
    for i in range(N_CORES):
        b = i // 2
        c0 = (i % 2) * GC
        q_sl = slice(c0, c0 + GC)
        k_sl = slice(D + c0, D + c0 + GC)
        v_sl = slice(2 * D + c0, 2 * D + c0 + GC)
        import ml_dtypes
        bf = ml_dtypes.bfloat16
        in_maps.append({
            "xt": np.ascontiguousarray(x[b].T.astype(bf)),
            "wqk": np.ascontiguousarray(np.concatenate(
                [w_qkv[:, q_sl], w_qkv[:, k_sl]], axis=1).astype(bf)),
            "wv": np.ascontiguousarray(w_qkv[:, v_sl].astype(bf)),
            "bqk": np.ascontiguousarray(
                np.concatenate([b_qkv[q_sl], b_qkv[k_sl]])),
            "wout": np.ascontiguousarray(w_out[c0:c0 + GC, :].astype(bf)),
        })
    return in_maps


def finish_output(res, x, w_qkv, b_qkv, w_out, b_out):
    b_qkv = np.asarray(b_qkv, dtype=np.float32)
    w_out = np.asarray(w_out, dtype=np.float32)
    b_out = np.asarray(b_out, dtype=np.float32)
    # V-projection bias passes through the softmax average; fold it into the
    # output bias: y += b_v @ w_out + b_out
    b_eff = b_qkv[2 * D:] @ w_out + b_out
    out = np.empty((B, S, D), dtype=np.float32)
    for b in range(B):
        # y comes back transposed [D, S]; un-transpose while summing the
        # two head-group partials
        out[b] = (res.results[2 * b]["y"].astype(np.float32)
                  + res.results[2 * b + 1]["y"].astype(np.float32)).T + b_eff
    return out


def kernel(x, w_qkv, b_qkv, w_out, b_out):
    global LAST_RESULTS
    in_maps = make_in_maps(x, w_qkv, b_qkv, w_out, b_out)
    nc = _get_nc()
    res = run_bass_kernel_spmd(nc, in_maps, core_ids=list(range(N_CORES)))
    LAST_RESULTS = res
    return finish_output(res, x, w_qkv, b_qkv, w_out, b_out)



# revision 64
# speedup vs baseline: 1.2971x; 1.0872x over previous
"""Multi-head attention (B=4, S=2048, D=768, H=12) on 8 TRN2 NeuronCores.

Sharding: core i -> batch i//2, heads 6*(i%2) .. 6*(i%2)+6 (48 (b,h) pairs,
6 per core). Each core computes q^T/k^T in [d, s] layout, V in natural
[s, d] layout (bf16, with an appended ones-column so the softmax denominator
falls out of the attnV matmul), transposed scores S^T[k, q], exp on the
scalar engine (bf16 out), then the partial output projection over its 384
attention-output channels. The two cores sharing a batch have their partial
projections summed host-side, which stands in for the tensor-parallel
all-reduce.

Performance structure:
- ALL matmul operands bf16 (xt/wqk/wv/qkT/V/expS/attnT/wout): the HW f32r
  matmul path measured far below its cost-model rate (qkT bf16 alone was
  -13us, the outproj f32r->bf16 transpose -58us). Paired score matmuls in
  64-row groups; attention inner loop software-pipelined (attnV trails the
  exp stream by TRAIL k-blocks; divisions of head pair j run inside pair
  j+1's loop).
- Attention processed per (head-pair, 512-wide q-quarter) so every psum
  tile is one bank: 6 rotating score tiles + 2 attnV accumulators. HW
  cross-engine dependency latency is far above the cost model's, so deep
  score rotation (3 k-blocks of slack on the scores->exp handoff) is worth
  more than big tiles (HW-swept: 6/2 = 434us vs 4/4 = 465us).
- exp split across engines: each k-block's TWO head-scores exp in ONE
  instruction (adjacent expS slots; halves the handoff count); odd
  k-blocks >= 3 on the DVE via Schraudolph int16-bitcast (A_SCH*s + B_SCH
  -> bf16 bits), the rest on ACT (table exp), keeping ACT under the PE's
  per-k-block budget at ~+0.8% attention-output error (gate is 2e-2).
- Output projection transposed (y^T[o,s] = wout^T @ attnT) with bf16
  wout stationary and bf16 attnT moving in all-512 chunks: the f32r
  outproj path (256-wide moving chunks / f32r operands) ran ~3x slower
  than modeled on HW (-58us measured); host un-transposes while summing
  the partials.
- Softmax denominator via an appended ones-column in V; normalize chain
  reciprocal (DVE) -> partition_broadcast (gpsimd) -> multiply (DVE), no
  PE matmul, issued two k-blocks early through the carry pipeline.
- x^T / w_qkv / w_v inputs in bf16: halves per-rep input DMA (9.8 -> 4.9 MB)
  and SBUF footprint; V/expS tiles bf16. Input DMAs split across both
  HWDGE queues (SP+ACT), first chunks small so the first matmul starts
  ~2us earlier; exp-table warm + bqk load issued after the critical DMAs.
- V-projection bias folded to the host (softmax rows sum to 1):
  y += b_v @ w_out + b_out in finish_output().
- Phase-B psum pool at bufs=8; phase D has its own 4-buffer psum pool and
  runs sb 8-15 first so the last pairs' div chains hide behind it; output
  staging + y DMA in bf16 (partials upcast to fp32 on the host).
- Timing methodology: wall-clock through the axon tunnel is dominated by
  per-call payload (~30MB/s, +-0.3s jitter), so exec time is measured with
  _build(hw_reps=K): a sequencer For_i loop around the body (constant NEFF
  size); (t[K=4001] - t[K=1]) / 4000 resolves per-rep time to a few us
  (bench3.py).
"""

import os

import numpy as np

import concourse.bass as bass
from concourse import bacc
import concourse.mybir as mybir
import concourse.tile as tile
from concourse.bass_utils import run_bass_kernel_spmd

F32 = mybir.dt.float32
F32R = mybir.dt.float32r
BF16 = mybir.dt.bfloat16
I16 = mybir.dt.int16
AF = mybir.ActivationFunctionType
ALU = mybir.AluOpType

B, S, D = 4, 2048, 768
H, HD = 12, 64
HPC = 6            # heads per core
GC = HPC * HD      # 384 channels per core
N_CORES = 8
SCALE = 1.0 / np.sqrt(np.float32(H))   # NOTE: reference scales by 1/sqrt(H)

# Schraudolph constants: bits = A_SCH * s_raw + B_SCH, int16 -> bitcast bf16
A_SCH = float(128.0 * np.log2(np.e) * SCALE)
B_SCH = float(128.0 * 127.0 - 7.5)

SPLIT_ATTNV = False
SPLIT_PROJ = False
SPLIT_OUTPROJ = False
# k-blocks whose (merged two-head) exp runs on the DVE (Schraudolph)
# instead of the scalar engine: unloads ACT (the phase-C co-critical
# engine) at ~1.2% extra attention-output error (validated in numpy; gate
# is 2e-2). Odd k-blocks >= 3: kb 0-2 stay on ACT so the DVE is clear of
# the div-carry burst (reciprocal + norm) at the start of each pair.
# kb 15 on ACT too (not DVE): frees the DVE right before the unit-boundary
# carry burst; with TRAIL=5, HW-measured 334.5us vs 363.7us for the 7-block
# TRAIL=4 config.
DVE_KBS = frozenset({3, 5, 7, 9, 11, 13})
if os.environ.get("KNOB_DVE_KBS") is not None:
    _v = os.environ["KNOB_DVE_KBS"]
    DVE_KBS = frozenset(int(x) for x in _v.split(",") if x != "")
SKIP = frozenset()
# psum split: 3 two-bank score tiles (one per k-block holding BOTH heads,
# 3-kb rotation slack on the scores->exp handoff, the dominant HW stall)
# + 2 single-bank attnV accumulators. Predecessor sweep (per-head tiles):
# 4/4=465us, 5/3=441us, 6/2=434us on the For_i paired bench.
PSS_BUFS = int(os.environ.get("KNOB_PSS_BUFS", "3"))
PSO_BUFS = int(os.environ.get("KNOB_PSO_BUFS", "2"))

_NC_CACHE = None
LAST_RESULTS = None


def _build(reps=1, hw_reps=1):
    """reps: python-unrolled repetitions (NEFF grows per rep).
    hw_reps: sequencer-level For_i loop around the body (constant NEFF size;
    used for tunnel-immune on-device timing via large iteration counts)."""
    nc = bacc.Bacc("TRN2", target_bir_lowering=False, debug=False,
                   num_devices=N_CORES)
    xt = nc.dram_tensor("xt", (D, S), BF16, kind="ExternalInput")
    wqk = nc.dram_tensor("wqk", (D, 2 * GC), BF16, kind="ExternalInput")
    wv = nc.dram_tensor("wv", (D, GC), BF16, kind="ExternalInput")
    bqk = nc.dram_tensor("bqk", (2 * GC,), F32, kind="ExternalInput")
    wout = nc.dram_tensor("wout", (GC, D), BF16, kind="ExternalInput")
    # y stored transposed [D, S]; the host sums the two per-batch partials
    # anyway, so it un-transposes for free in finish_output
    y = nc.dram_tensor("y", (D, S), BF16, kind="ExternalOutput")

    NSB = S // 128        # 16 s-blocks
    NDS = D // 128        # 6 d-subtiles
    NOB = 2 * GC // 128   # 6 q+k output blocks
    NCS = GC // 128       # 3 c-subtiles for out-proj

    with tile.TileContext(nc) as tc:
        with (
            tc.tile_pool(name="const", bufs=1) as cpool,
            tc.tile_pool(name="bigqv", bufs=1) as bigqv,
            tc.tile_pool(name="bigd", bufs=1) as bigd,
        ):
            ones_sb = cpool.tile([1, 128], F32)
            nc.gpsimd.memset(ones_sb[:], 1.0)
            ones_r = cpool.tile([1, 128], F32R)
            nc.vector.tensor_copy(ones_r[:], ones_sb[:])
            # exp ACT table preload + bqk load are emitted inside rep_body
            # AFTER the phase-B input-DMA dispatches: both sit on queues
            # (ACT/SP) whose first dma_starts gate the very first matmul
            warm = cpool.tile([1, 8], F32)
            bqk_sb = cpool.tile([128, NOB], F32)
            wout_sb = cpool.tile([128, NCS, D], BF16)

            def warm_and_bqk():
                nc.sync.dma_start(
                    bqk_sb[:], bqk.ap().rearrange("(ob p) -> p ob", p=128))
                nc.scalar.activation(warm[:], ones_sb[:, :8], AF.Exp)

            # bf16 q^T/k^T: avoids the HW f32r matmul-path penalty found on
            # the output projection, and halves the tile (48 -> 24 KB/part)
            qkT = bigqv.tile([128, NOB, S], BF16)     # blocks 0-2 q^T, 3-5 k^T
            V_sb = bigqv.tile([128, NSB, HPC * (HD + 1)], BF16)  # V + ones col
            attnT = bigd.tile([128, NCS, S], BF16)    # attention out, [c, s]

            if "div" in SKIP:     # ablation builds: attnT must have a writer
                nc.vector.memset(attnT[:], 0.0)

            V_view = V_sb[:].rearrange("p b (h e) -> p b h e", e=HD + 1)
            ones_col = cpool.tile([128, 1], BF16)
            nc.gpsimd.memset(ones_col[:], 1.0)
            nc.vector.tensor_copy(
                V_view[:, :, :, HD], ones_col[:, :, None].to_broadcast([128, NSB, HPC])
            )

            xt_src = xt.ap().rearrange("(ds p) s -> p ds s", p=128)
            wqk_src = wqk.ap().rearrange("(ds p) o -> p ds o", p=128)

            rep_ctx = tc.tile_pool(name="xtp", bufs=2)
            xtp = rep_ctx.__enter__()

            def rep_body(load_wout):
                # ---- Phase B: projections ----
                OB_ORDER = (0, 3, 1, 4, 2, 5)
                with (
                    tc.tile_pool(name="psb", bufs=8, space="PSUM") as psb,
                ):
                    xt_sb = xtp.tile([128, NDS, S], BF16, tag="xt",
                                     name="xt_sb")
                    wqk_sb = xtp.tile([128, NDS, 2 * GC], BF16, tag="wq",
                                      name="wqk_sb")
                    wv_sb = xtp.tile([128, NDS, GC], BF16, tag="wv",
                                     name="wv_sb")
                    # input DMAs split across the two HWDGE queues (SP+ACT,
                    # ~600ns descriptor-gen each, serial per queue); xt's
                    # first chunk in ds-halves so the first ob-0 matmuls
                    # (ds-major) start as early as possible
                    nc.scalar.dma_start(
                        wqk_sb[:, :, 0:128], wqk_src[:, :, 0:128],
                    )
                    for d0, d1 in ((0, 3), (3, 6)):
                        nc.sync.dma_start(
                            xt_sb[:, d0:d1, 0:512],
                            xt_src[:, d0:d1, 0:512],
                        )
                    nc.scalar.dma_start(
                        xt_sb[:, :, 512:1024], xt_src[:, :, 512:1024],
                    )
                    nc.scalar.dma_start(
                        wqk_sb[:, :, 3 * 128:4 * 128],
                        wqk_src[:, :, 3 * 128:4 * 128],
                    )
                    for sc, eng in ((2, nc.sync), (3, nc.scalar)):
                        eng.dma_start(
                            xt_sb[:, :, sc * 512:(sc + 1) * 512],
                            xt_src[:, :, sc * 512:(sc + 1) * 512],
                        )
                    if load_wout:
                        warm_and_bqk()
                    for ob in (1, 4, 2, 5):
                        nc.sync.dma_start(
                            wqk_sb[:, :, ob * 128:(ob + 1) * 128],
                            wqk_src[:, :, ob * 128:(ob + 1) * 128],
                        )
                    nc.sync.dma_start(
                        wv_sb[:],
                        wv.ap().rearrange("(ds p) o -> p ds o", p=128),
                    )
                    if load_wout:
                        nc.sync.dma_start(
                            wout_sb[:],
                            wout.ap().rearrange("(cs p) o -> p cs o", p=128),
                        )

                    # q^T / k^T: [o, s] = wqk^T @ x^T
                    halves = ((0, 64), (64, 128)) if SPLIT_PROJ else ((0, 128),)
                    for ob in OB_ORDER:
                        pss4 = [psb.tile([128, 512], F32, tag="ps",
                                         name=f"ps{ob}_{sc}") for sc in range(4)]
                        # ob 0 runs sc-outer: its first 6 matmuls then need
                        # only the first xt chunk, which lands ~3us before
                        # the rest of xt
                        if ob == 0:
                            loop_iter = [(ds, sc) for sc in range(4)
                                         for ds in range(NDS)]
                        else:
                            loop_iter = [(ds, sc) for ds in range(NDS)
                                         for sc in range(4)]
                        for ds, sc in loop_iter if "proj" not in SKIP else ():
                            for hi, (r0, r1) in enumerate(halves):
                                nc.tensor.matmul(
                                    pss4[sc][:],
                                    wqk_sb[r0:r1, ds, ob * 128:(ob + 1) * 128],
                                    xt_sb[r0:r1, ds, sc * 512:(sc + 1) * 512],
                                    start=(ds == 0 and hi == 0),
                                    stop=(ds == NDS - 1
                                          and hi == len(halves) - 1),
                                    skip_group_check=SPLIT_PROJ,
                                )
                        for sc in range(4):
                            nc.vector.tensor_scalar_add(
                                qkT[:, ob, sc * 512:(sc + 1) * 512], pss4[sc][:],
                                bqk_sb[:, ob:ob + 1],
                            )

                    # V natural: [s, o] = x @ wv   (bias folded to host)
                    for sb in range(NSB):
                        ps = psb.tile([128, 512], F32, tag="ps")
                        for ds in range(NDS) if "proj" not in SKIP else ():
                            for hi, (r0, r1) in enumerate(halves):
                                nc.tensor.matmul(
                                    ps[:, :GC],
                                    xt_sb[r0:r1, ds, sb * 128:(sb + 1) * 128],
                                    wv_sb[r0:r1, ds, :],
                                    start=(ds == 0 and hi == 0),
                                    stop=(ds == NDS - 1
                                          and hi == len(halves) - 1),
                                    skip_group_check=True,
                                )
                        nc.vector.tensor_copy(V_view[:, sb, :, 0:HD], ps[:, :GC])

                # ---- Phase C: attention per (head-pair, q-quarter) ----
                # q processed in 512-wide quarters so every psum tile is a
                # single bank: pss 4 bufs + pso 4 bufs fills the 8 banks and
                # doubles the rotation slack on every cross-engine handoff
                # (HW dependency latency is far above the cost model's 100ns
                # -- measured via SKIP ablations on the For_i bench).
                with (
                    tc.tile_pool(name="bigc", bufs=1) as bigc,
                    tc.tile_pool(name="cw", bufs=1) as cw,
                    tc.tile_pool(name="pss", bufs=PSS_BUFS, space="PSUM") as pss,
                    tc.tile_pool(name="pso", bufs=PSO_BUFS, space="PSUM") as pso,
                ):
                    QQ = 512
                    NQQ = S // QQ  # 4 q-quarters
                    NBUF = 12      # rotating S^T exp slots (2 per k-block)
                    expS = bigc.tile([128, NBUF, QQ], BF16)

                    def slot(kb, hi):
                        return (2 * kb + hi) % NBUF

                    def make_div(ps_o, h, qq):
                        # normalize out'[d, q] by Z[q] (ones-column row).
                        # reciprocal [1,QQ] (DVE, from psum), partition-
                        # broadcast (gpsimd), multiply (DVE): no PE matmul,
                        # no shared-psum-pool collision with the score tiles.
                        # (HW-tested alternatives all regressed: PE rank-1
                        # broadcast via a borrowed score-pool tile +70us,
                        # norm muls on Pool +sim-visible stalls.)
                        if "div" in SKIP:
                            return lambda: None, lambda: None
                        base = (h % 2) * 64
                        qob = h // 2
                        rz = cw.tile([1, QQ], F32, tag="rz", bufs=2, name="rz")
                        rzb_sb = cw.tile([64, QQ], F32, tag="rzb", bufs=2,
                                         name="rzb_sb")

                        def recip_bcast():
                            with nc.allow_low_precision(reason="f32r softmax denom"):
                                nc.vector.reciprocal(rz[:], ps_o[HD:HD + 1, :])
                            nc.gpsimd.partition_broadcast(rzb_sb[:], rz[:])

                        def norm():
                            nc.vector.tensor_mul(
                                attnT[base:base + 64, qob, qq * QQ:(qq + 1) * QQ],
                                ps_o[0:HD, :], rzb_sb[:],
                            )
                        return recip_bcast, norm

                    vhalves = ((0, 64), (64, 128)) if SPLIT_ATTNV else ((0, 128),)

                    def attn_v(ps_o, h, kb, start, stop):
                        if "attnv" in SKIP:
                            return
                        sl = slot(kb, h % 2)
                        for hi, (r0, r1) in enumerate(vhalves):
                            nc.tensor.matmul(
                                ps_o[:, :],
                                V_sb[r0:r1, kb,
                                     h * (HD + 1):(h + 1) * (HD + 1)],
                                expS[r0:r1, sl, :],
                                start=(start and hi == 0),
                                stop=(stop and hi == len(vhalves) - 1),
                                skip_group_check=True,
                            )

                    def do_exp(ps_s, kb):
                        # ONE instruction covers both heads' scores: the two
                        # expS slots for (kb,0)/(kb,1) are adjacent (slot of
                        # (kb,0) is always even), halving the exp instruction
                        # and cross-engine handoff count per k-block
                        if "exp" in SKIP:
                            return
                        sl = slot(kb, 0)
                        if kb in DVE_KBS:
                            nc.vector.tensor_scalar(
                                expS[:, sl:sl + 2, :].bitcast(I16), ps_s[:],
                                A_SCH, B_SCH, ALU.mult, ALU.add,
                            )
                        else:
                            nc.scalar.activation(
                                expS[:, sl:sl + 2, :], ps_s[:], AF.Exp,
                                scale=float(SCALE),
                            )

                    TRAIL = 5
                    carry = []   # closures from the previous (pair, qq)
                    # qq 2,3 first: phase D then starts with sb 8-15
                    # (ready long before), hiding the last pairs' div chains
                    for qq in (2, 3, 0, 1):
                        for hp in range(HPC // 2):
                            hA, hB = 2 * hp, 2 * hp + 1
                            qob = hp
                            kob = NCS + hp
                            ps_oA = pso.tile([HD + 1, QQ], F32, tag="o", name="ps_oA")
                            ps_oB = pso.tile([HD + 1, QQ], F32, tag="o", name="ps_oB")
                            for kb in range(NSB):
                                ps_s = pss.tile([128, 2, QQ], F32, tag="s",
                                                name="ps_s")
                                for hi, base in ((0, 0), (1, 64)) \
                                        if "scores" not in SKIP else ():
                                    nc.tensor.matmul(
                                        ps_s[:, hi, :],
                                        qkT[base:base + 64, kob,
                                            kb * 128:(kb + 1) * 128],
                                        qkT[base:base + 64, qob,
                                            qq * QQ:(qq + 1) * QQ],
                                        start=True, stop=True,
                                    )
                                do_exp(ps_s, kb)
                                if kb < len(carry):
                                    carry[kb]()
                                if kb >= TRAIL:
                                    pk = kb - TRAIL
                                    attn_v(ps_oA, hA, pk, start=(pk == 0), stop=False)
                                    attn_v(ps_oB, hB, pk, start=(pk == 0), stop=False)
                            for pk in range(NSB - TRAIL, NSB - 1):
                                attn_v(ps_oA, hA, pk, start=False, stop=False)
                                attn_v(ps_oB, hB, pk, start=False, stop=False)
                            rbA, normA = make_div(ps_oA, hA, qq)
                            rbB, normB = make_div(ps_oB, hB, qq)
                            carry = [
                                lambda a=ps_oA, b=ps_oB, h1=hA, h2=hB, \
                                        rA=rbA, rB=rbB: (
                                    attn_v(a, h1, NSB - 1, start=False, stop=True),
                                    attn_v(b, h2, NSB - 1, start=False, stop=True),
                                    rA(), rB(),
                                ),
                                lambda nA=normA, nB=normB: (nA(), nB()),
                            ]
                    for f in carry:
                        f()

                # ---- Phase D: output projection (partial, 384 c) ----
                # transposed: y^T[o, s] = wout^T @ attnT with wout (bf16) as
                # the stationary operand and attnT (f32r) moving in all-512
                # chunks -- no 256-wide f32r moving chunks (rate-sensitive on
                # HW) and no f32r stationary loads. Own psum pool (phase C's
                # pools are closed); q-chunks 2,3 first (normalized pairs
                # ago) so the last pairs' div chains hide behind them.
                with (
                    tc.tile_pool(name="psd", bufs=6, space="PSUM") as psd,
                    tc.tile_pool(name="cwd", bufs=1) as cwd,
                ):
                    y_dst = y.ap().rearrange("(ob p) s -> p ob s", p=128)
                    for sc in (2, 3, 0, 1):
                        for ob in range(D // 128):
                            ps_f = psd.tile([128, QQ], F32, tag="f",
                                            name="ps_f")
                            for cs in range(NCS) if "outproj" not in SKIP else ():
                                nc.tensor.matmul(
                                    ps_f[:],
                                    wout_sb[:, cs, ob * 128:(ob + 1) * 128],
                                    attnT[:, cs, sc * QQ:(sc + 1) * QQ],
                                    start=(cs == 0), stop=(cs == NCS - 1),
                                )
                            if "outproj" not in SKIP:
                                ostage = cwd.tile([128, QQ], BF16,
                                                  tag="ostage", bufs=4,
                                                  name="ostage")
                                # ACT is idle during phase D; keep the
                                # psum->sbuf staging off the DVE
                                nc.scalar.activation(ostage[:], ps_f[:],
                                                     AF.Copy)
                                nc.sync.dma_start(
                                    y_dst[:, ob, sc * QQ:(sc + 1) * QQ],
                                    ostage[:])

            if hw_reps > 1:
                warm_and_bqk()
                nc.sync.dma_start(
                    wout_sb[:],
                    wout.ap().rearrange("(cs p) o -> p cs o", p=128),
                )
                with tc.For_i(0, hw_reps):
                    rep_body(load_wout=False)
            else:
                for _rep in range(reps):
                    rep_body(load_wout=(_rep == 0))
            rep_ctx.__exit__(None, None, None)

    nc.compile()
    return nc


def _get_nc():
    global _NC_CACHE
    if _NC_CACHE is None:
        _NC_CACHE = _build()
    return _NC_CACHE


def make_in_maps(x, w_qkv, b_qkv, w_out, b_out):
    x = np.asarray(x, dtype=np.float32)
    w_qkv = np.asarray(w_qkv, dtype=np.float32)
    b_qkv = np.asarray(b_qkv, dtype=np.float32)
    w_out = np.asarray(w_out, dtype=np.float32)

    in_maps = []
    for i in range(N_CORES):
        b = i // 2
        c0 = (i % 2) * GC
        q_sl = slice(c0, c0 + GC)
        k_sl = slice(D + c0, D + c0 + GC)
        v_sl = slice(2 * D + c0, 2 * D + c0 + GC)
        import ml_dtypes
        bf = ml_dtypes.bfloat16
        in_maps.append({
            "xt": np.ascontiguousarray(x[b].T.astype(bf)),
            "wqk": np.ascontiguousarray(np.concatenate(
                [w_qkv[:, q_sl], w_qkv[:, k_sl]], axis=1).astype(bf)),
            "wv": np.ascontiguousarray(w_qkv[:, v_sl].astype(bf)),
            "bqk": np.ascontiguousarray(
                np.concatenate([b_qkv[q_sl], b_qkv[k_sl]])),
            "wout": np.ascontiguousarray(w_out[c0:c0 + GC, :].astype(bf)),
        })
    return in_maps


def finish_output(res, x, w_qkv, b_qkv, w_out, b_out):
    b_qkv = np.asarray(b_qkv, dtype=np.float32)
    w_out = np.asarray(w_out, dtype=np.float32)
    b_out = np.asarray(b_out, dtype=np.float32)
    # V-projection bias passes through the softmax average; fold it into the
    # output bias: y += b_v @ w_out + b_out
    b_eff = b_qkv[2 * D:] @ w_out + b_out
    out = np.empty((B, S, D), dtype=np.float32)
    for b in range(B):
        # y comes back transposed [D, S]; un-transpose while summing the
        # two head-group partials
        out[b] = (res.results[2 * b]["y"].astype(np.float32)
                  + res.results[2 * b + 1]["y"].astype(np.float32)).T + b_eff
    return out


def kernel(x, w_qkv, b_qkv, w_out, b_out):
    global LAST_RESULTS
    in_maps = make_in_maps(x, w_qkv, b_qkv, w_out, b_out)
    nc = _get_nc()
    res = run_bass_kernel_spmd(nc, in_maps, core_ids=list(range(N_CORES)))
    LAST_RESULTS = res
    return finish_output(res, x, w_qkv, b_qkv, w_out, b_out)

